# revision 1
# baseline (speedup 1.0000x reference)
"""CTC loss (Keras ctc_batch_cost semantics) on 8 Trainium2 NeuronCores.

Strategy: pure data parallel over batch (256 examples per core).

Per core:
- y_pred shard is viewed as [BC*16, T*V/16] f32: example e owns 16 consecutive
  partition rows; row j holds frames t in [16j, 16j+16) (contiguous DMA slabs).
- The per-example extended-label gather p[e, t, ext[e, s]] runs on GPSIMD
  indirect_copy (one 16-partition group per example, shared index stream),
  split in two (ISA limit: <=1024 output elems per instruction).
- ScalarE rescales gathered probs: p*128 + 128*eps (folds CTC's +eps and keeps
  the linear-space DP near 1.0).
- DMA repacks gather output into pbuf[128 part = example-within-half, t-plane,
  2 batch-half groups, 67 (2 pads + 65 states)].
- VectorE runs the linear-space CTC forward DP over t (4 tensor ops per step)
  with periodic max-renormalization; ScalarE accumulates ln(renorm).
- loss = -(ln(a[S-1] + a[S-2]) + sum(ln renorm) - T*ln(128)).
"""
import sys

sys.path.insert(0, "/opt/trn_rl_repo")

import numpy as np
import concourse.bacc as bacc
import concourse.mybir as mybir
import concourse.tile as tile
from concourse.bass_utils import run_bass_kernel_spmd

F32 = mybir.dt.float32
U16 = mybir.dt.uint16
ADD = mybir.AluOpType.add
MULT = mybir.AluOpType.mult

B, T, L, V = 2048, 256, 32, 128
NCORES = 8
BC = B // NCORES           # 256 examples per core
BLANK = V - 1
EPS = 1e-7
SCALE = 128.0
LN_SCALE = float(np.log(SCALE))
S = 2 * L + 1              # 65
SPAD = S + 2               # 2 front pads
NT = BC // 8               # example tiles (8 examples x 16 partitions)
PH = BC // 2               # partitions used by the DP (two groups per partition)
TB = T // 16               # frame rows per partition slab
NSPLIT = -(-(TB * S) // 1024)
TBS = TB // NSPLIT
NIDX = TBS * S             # gather stream length per split
WPT = -(-NIDX // 16) + ((-(-NIDX // 16)) & 1)  # idx words/tile, 4B-aligned
SLAB = TB * V
SLABS = SLAB // NSPLIT
RENORM = 8

_NC_CACHE = {}


def _host_prep_core(y_true_core):
    """Aux tensors from one core's labels [BC, L] -> idx u16, cst f32."""
    ext = np.full((BC, S), BLANK, np.int64)
    ext[:, 1::2] = y_true_core
    ext_m2 = np.concatenate([np.full((BC, 2), -1, np.int64), ext[:, : S - 2]], 1)
    skip = (ext != BLANK) & (ext != ext_m2)
    mask2 = np.zeros((BC, S), np.float32)
    mask2[:, : S - 2] = skip[:, 2:].astype(np.float32)

    idx = np.zeros((128, NT * WPT), np.uint16)
    tau = np.repeat(np.arange(TBS), S)
    ss = np.tile(np.arange(S), TBS)
    stream = np.zeros(WPT * 16, np.int64)
    for i in range(NT):
        for g8 in range(8):
            e = 8 * i + g8 if i < NT // 2 else PH + 8 * (i - NT // 2) + g8
            stream[:] = 0
            stream[:NIDX] = tau * V + ext[e, ss]
            idx[16 * g8 : 16 * g8 + 16, i * WPT : (i + 1) * WPT] = (
                stream.reshape(WPT, 16).T
            )

    cst = np.zeros((128, 260), np.float32)
    for e in range(BC):
        p, g = (e, 0) if e < PH else (e - PH, 1)
        cst[p, g * S : g * S + S] = mask2[e]
    cst[:, 130:132] = 1.0
    cst[:, 195:197] = 1.0
    return idx, cst


def _build_nc(repeat=1):
    nc = bacc.Bacc()
    yp = nc.dram_tensor("yp", [BC * 16, SLAB], F32, kind="ExternalInput")
    idx_d = nc.dram_tensor("idx", [128, NT * WPT], U16, kind="ExternalInput")
    cst_d = nc.dram_tensor("cst", [128, 260], F32, kind="ExternalInput")
    loss_d = nc.dram_tensor("loss", [128, 2], F32, kind="ExternalOutput")

    with tile.TileContext(nc) as tc:
        import contextlib
        rep = tc.For_i(0, repeat, 1, name="rep") if repeat > 1 else contextlib.nullcontext()
        with (
            rep,
            tc.tile_pool(name="const", bufs=1) as constp,
            tc.tile_pool(name="pbuf", bufs=1) as pbufp,
            tc.tile_pool(name="raw", bufs=2) as rawp,
            tc.tile_pool(name="gat", bufs=2) as gatp,
            tc.tile_pool(name="state", bufs=1) as statep,
        ):
            idx_t = constp.tile([128, NT * WPT], U16)
            nc.sync.dma_start(idx_t[:], idx_d[:])
            cst_t = constp.tile([128, 260], F32)
            nc.sync.dma_start(cst_t[:], cst_d[:])
            mask2v = cst_t[:, 0:130].rearrange("p (g s) -> p g s", g=2)
            e01v = cst_t[:, 130:260].rearrange("p (g s) -> p g s", g=2)

            pbuf = pbufp.tile([128, T, 2, SPAD], F32)

            for i in range(NT):
                raw = rawp.tile([128, SLAB], F32, tag="raw")
                nc.sync.dma_start(raw[:], yp[i * 128 : (i + 1) * 128, :])
                G = gatp.tile([128, NSPLIT * NIDX], F32, tag="G")
                for sp in range(NSPLIT):
                    nc.gpsimd.indirect_copy(
                        G[:, sp * NIDX : (sp + 1) * NIDX],
                        raw[:, sp * SLABS : (sp + 1) * SLABS],
                        idx_t[:, i * WPT : (i + 1) * WPT],
                        True,
                    )
                nc.scalar.activation(
                    G[:], G[:], mybir.ActivationFunctionType.Copy,
                    bias=SCALE * EPS, scale=SCALE,
                )
                p0 = 8 * i if i < NT // 2 else 8 * (i - NT // 2)
                gc = 0 if i < NT // 2 else 1
                Gv = G[:].rearrange("p (tau s) -> p tau s", s=S)
                for g8 in range(8):
                    nc.sync.dma_start(
                        pbuf[p0 + g8 : p0 + g8 + 1, :, gc, 2 : 2 + S],
                        Gv[16 * g8 : 16 * g8 + 16],
                    )

            x = statep.tile([128, 2, SPAD], F32)
            am = statep.tile([128, 2, SPAD], F32)
            u = statep.tile([128, 2, S], F32)
            w = statep.tile([128, 2, S], F32)
            m = statep.tile([128, 2], F32)
            rec = statep.tile([128, 2], F32)
            lacc = statep.tile([128, 2], F32)
            lnm = statep.tile([128, 2], F32)
            s2 = statep.tile([128, 2], F32)
            lossT = statep.tile([128, 2], F32)

            nc.vector.memset(x[:], 0.0)
            nc.vector.memset(am[:], 0.0)
            nc.vector.memset(lacc[:], 0.0)

            xr = x[0:PH, :, 2 : 2 + S]
            xm1 = x[0:PH, :, 1 : 1 + S]
            amm2 = am[0:PH, :, 0:S]
            amr = am[0:PH, :, 2 : 2 + S]
            m2v = mask2v[0:PH]
            uv = u[0:PH, :, :]
            wv = w[0:PH, :, :]
            mv = m[0:PH, :]
            recv = rec[0:PH, :]
            laccv = lacc[0:PH, :]
            lnmv = lnm[0:PH, :]

            nc.vector.tensor_tensor(xr, pbuf[0:PH, 0, :, 2 : 2 + S], e01v[0:PH], MULT)
            nc.vector.tensor_tensor(amr, xr, m2v, MULT)

            for t in range(1, T):
                pv = pbuf[0:PH, t, :, 2 : 2 + S]
                nc.vector.tensor_tensor(uv, xr, xm1, ADD)
                nc.vector.tensor_tensor(wv, uv, amm2, ADD)
                nc.vector.tensor_tensor(xr, wv, pv, MULT)
                if t % RENORM == 0:
                    nc.vector.reduce_max(mv, xr, axis=mybir.AxisListType.X)
                    nc.vector.reciprocal(recv, mv)
                    recb = recv.unsqueeze(2).broadcast_to((PH, 2, S))
                    nc.vector.tensor_tensor(xr, xr, recb, MULT)
                    nc.scalar.activation(lnmv, mv, mybir.ActivationFunctionType.Ln)
                    nc.vector.tensor_tensor(laccv, laccv, lnmv, ADD)
                nc.vector.tensor_tensor(amr, xr, m2v, MULT)

            nc.vector.tensor_tensor(
                s2[0:PH, :],
                x[0:PH, :, 1 + S : 2 + S].rearrange("p g one -> p (g one)"),
                x[0:PH, :, S : 1 + S].rearrange("p g one -> p (g one)"),
                ADD,
            )
            nc.scalar.activation(lnmv, s2[0:PH, :], mybir.ActivationFunctionType.Ln)
            nc.vector.tensor_tensor(lnmv, lnmv, laccv, ADD)
            nc.scalar.activation(
                lossT[0:PH, :], lnmv, mybir.ActivationFunctionType.Copy,
                bias=T * LN_SCALE, scale=-1.0,
            )
            nc.sync.dma_start(loss_d[0:PH, :], lossT[0:PH, :])

    nc.compile()
    return nc


def kernel(y_true, y_pred, _trace=False):
    y_true = np.asarray(y_true)
    y_pred = np.ascontiguousarray(np.asarray(y_pred, dtype=np.float32))
    assert y_true.shape == (B, L) and y_pred.shape == (B, T, V)

    if "nc" not in _NC_CACHE:
        _NC_CACHE["nc"] = _build_nc()
    nc = _NC_CACHE["nc"]

    in_maps = []
    for c in range(NCORES):
        idx, cst = _host_prep_core(np.asarray(y_true[c * BC : (c + 1) * BC]))
        in_maps.append(
            {
                "yp": y_pred[c * BC : (c + 1) * BC].reshape(BC * 16, SLAB),
                "idx": idx,
                "cst": cst,
            }
        )

    res = run_bass_kernel_spmd(nc, in_maps, core_ids=list(range(NCORES)), trace=_trace)

    out = np.empty((B, 1), np.float32)
    for c in range(NCORES):
        lo = res.results[c]["loss"]
        out[c * BC : c * BC + PH, 0] = lo[:PH, 0]
        out[c * BC + PH : (c + 1) * BC, 0] = lo[:PH, 1]
    if _trace:
        return out, res
    return out

